# revision 1
# baseline (speedup 1.0000x reference)
"""Trainium2 Bass kernel for nn_Policy_11484742550172.

The reference pads each input channel with 100 zeros on the right and keeps
the last 32 columns — with 100 >= 32 the conv input is exactly zero for any
x, so the network collapses to a weights-only dense chain:

    v1 = relu(conv1_b)                                  [8]
    v2 = relu(sum_k conv2_w[:, :, k] @ v1 + conv2_b)    [16]
    v3 = relu(sum_k conv3_w[:, :, k] @ v2 + conv3_b)    [32]
    v4 = relu(conv4_w[:, :, 0] @ v3 + conv4_b)          [32]
    h   = relu(fc1_w.reshape(128, 32, 30).sum(-1) @ v4 + fc1_b)
    out = softmax(fc2_w @ h + fc2_b)
        = sigmoid([l0 - l1, l1 - l0])   (softmax over 2 = sigmoid of diff)

This is an exact algebraic simplification (conv of zeros = bias), not an
approximation. x and conv1_w never influence the output.

Sharding: the problem is far too small to shard; the kernel is replicated
SPMD on all 8 cores and core 0's output is returned.
"""

import numpy as np

import concourse.bass as bass
import concourse.tile as tile
from concourse import bacc, mybir
from concourse.bass_utils import run_bass_kernel_spmd

N_CORES = 8
F32 = mybir.dt.float32
ALU = mybir.AluOpType
ACT = mybir.ActivationFunctionType

_CACHE = {}


def _build():
    nc = bacc.Bacc(
        "TRN2",
        target_bir_lowering=False,
        debug=False,
        num_devices=N_CORES,
        enable_partition_id=False,
    )

    b1d = nc.dram_tensor("conv1_b", [8, 1], F32, kind="ExternalInput")
    w2d = nc.dram_tensor("conv2_w", [16, 8, 2], F32, kind="ExternalInput")
    b2d = nc.dram_tensor("conv2_b", [16, 1], F32, kind="ExternalInput")
    w3d = nc.dram_tensor("conv3_w", [32, 16, 2], F32, kind="ExternalInput")
    b3d = nc.dram_tensor("conv3_b", [32, 1], F32, kind="ExternalInput")
    w4d = nc.dram_tensor("conv4_w", [32, 32], F32, kind="ExternalInput")
    b4d = nc.dram_tensor("conv4_b", [32, 1], F32, kind="ExternalInput")
    fw1d = nc.dram_tensor("fc1_w", [128, 960], F32, kind="ExternalInput")
    fb1d = nc.dram_tensor("fc1_b", [128, 1], F32, kind="ExternalInput")
    fw2d = nc.dram_tensor("fc2_w", [2, 128], F32, kind="ExternalInput")
    fb2d = nc.dram_tensor("fc2_b", [1, 2], F32, kind="ExternalInput")
    outd = nc.dram_tensor("out", [1, 2], F32, kind="ExternalOutput")

    with tile.TileContext(nc) as tc:
        with (
            tc.tile_pool(name="sb", bufs=1) as sb,
            tc.tile_pool(name="ps", bufs=1, space="PSUM") as ps,
        ):
            zero = nc.const_aps.aps[(F32, 0.0)]
            one = nc.const_aps.aps[(F32, 1.0)]

            # Warm the sigmoid ACT table while DMAs are in flight.
            warm = sb.tile([1, 1], F32)
            nc.scalar.activation(warm[:], zero[:1, :1], ACT.Sigmoid)

            # --- small weight/bias loads (transposed views where needed) ---
            b1 = sb.tile([8, 1], F32)
            nc.gpsimd.dma_start(b1[:], b1d[:])
            w2 = sb.tile([8, 16, 2], F32)
            nc.gpsimd.dma_start(w2[:], w2d.ap().rearrange("o i k -> i o k"))
            b2 = sb.tile([16, 1], F32)
            nc.gpsimd.dma_start(b2[:], b2d[:])
            w3 = sb.tile([16, 32, 2], F32)
            nc.gpsimd.dma_start(w3[:], w3d.ap().rearrange("o i k -> i o k"))
            b3 = sb.tile([32, 1], F32)
            nc.gpsimd.dma_start(b3[:], b3d[:])
            w4 = sb.tile([32, 32], F32)
            nc.gpsimd.dma_start(w4[:], w4d.ap().rearrange("o i -> i o"))
            b4 = sb.tile([32, 1], F32)
            nc.gpsimd.dma_start(b4[:], b4d[:])
            fb1 = sb.tile([128, 1], F32)
            nc.gpsimd.dma_start(fb1[:], fb1d[:])
            fw2 = sb.tile([128, 2], F32)
            nc.gpsimd.dma_start(fw2[:], fw2d.ap().rearrange("o k -> k o"))
            fb2 = sb.tile([1, 2], F32)
            nc.gpsimd.dma_start(fb2[:], fb2d[:])

            # --- big fc1 weight load, 4 column chunks ---
            fw1 = sb.tile([128, 960], F32)
            for c in range(4):
                nc.gpsimd.dma_start(
                    fw1[:, c * 240 : (c + 1) * 240], fw1d[:, c * 240 : (c + 1) * 240]
                )

            # --- conv chain (input is all zeros -> bias-fed dense chain) ---
            v1 = sb.tile([8, 1], F32)
            nc.vector.tensor_scalar(v1[:], b1[:], 0.0, None, op0=ALU.max)

            w2s = sb.tile([8, 16], F32)
            nc.vector.tensor_reduce(
                out=w2s[:], in_=w2[:], axis=mybir.AxisListType.X, op=ALU.add
            )
            p2 = ps.tile([16, 1], F32)
            nc.tensor.matmul(p2[:], w2s[:], v1[:], start=True, stop=True)
            v2 = sb.tile([16, 1], F32)
            nc.vector.tensor_scalar(v2[:], p2[:], b2[:], 0.0, op0=ALU.add, op1=ALU.max)

            w3s = sb.tile([16, 32], F32)
            nc.vector.tensor_reduce(
                out=w3s[:], in_=w3[:], axis=mybir.AxisListType.X, op=ALU.add
            )
            p3 = ps.tile([32, 1], F32)
            nc.tensor.matmul(p3[:], w3s[:], v2[:], start=True, stop=True)
            v3 = sb.tile([32, 1], F32)
            nc.vector.tensor_scalar(v3[:], p3[:], b3[:], 0.0, op0=ALU.add, op1=ALU.max)

            p4 = ps.tile([32, 1], F32)
            nc.tensor.matmul(p4[:], w4[:], v3[:], start=True, stop=True)
            v4 = sb.tile([32, 1], F32)
            nc.vector.tensor_scalar(v4[:], p4[:], b4[:], 0.0, op0=ALU.add, op1=ALU.max)

            # --- fc1: group-sum fc1_w over the 30 repeated positions ---
            w1r = sb.tile([128, 32], F32)
            for c in range(4):
                nc.vector.tensor_reduce(
                    out=w1r[:, c * 8 : (c + 1) * 8],
                    in_=fw1[:, c * 240 : (c + 1) * 240].rearrange(
                        "p (o t) -> p o t", t=30
                    ),
                    axis=mybir.AxisListType.X,
                    op=ALU.add,
                )
            w1t = sb.tile([32, 128], F32)
            for c in range(4):
                nc.vector.transpose(
                    w1t[:, c * 32 : (c + 1) * 32], w1r[c * 32 : (c + 1) * 32, :]
                )

            py = ps.tile([128, 1], F32)
            nc.tensor.matmul(py[:], w1t[:], v4[:], start=True, stop=True)
            h = sb.tile([128, 1], F32)
            nc.vector.tensor_scalar(h[:], py[:], fb1[:], 0.0, op0=ALU.add, op1=ALU.max)

            # --- fc2 as a logit-difference matvec; softmax(2) = sigmoid ---
            dw = sb.tile([128, 2], F32)
            nc.vector.tensor_tensor(
                out=dw[:, 0:1], in0=fw2[:, 0:1], in1=fw2[:, 1:2], op=ALU.subtract
            )
            nc.vector.tensor_tensor(
                out=dw[:, 1:2], in0=fw2[:, 1:2], in1=fw2[:, 0:1], op=ALU.subtract
            )
            db = sb.tile([1, 2], F32)
            nc.vector.tensor_tensor(
                out=db[:, 0:1], in0=fb2[:, 0:1], in1=fb2[:, 1:2], op=ALU.subtract
            )
            nc.vector.tensor_tensor(
                out=db[:, 1:2], in0=fb2[:, 1:2], in1=fb2[:, 0:1], op=ALU.subtract
            )

            pl = ps.tile([1, 2], F32)
            nc.tensor.matmul(pl[:], h[:], dw[:], start=True, stop=False)
            nc.tensor.matmul(pl[:], one[:1, :1], db[:], start=False, stop=True)

            probs = sb.tile([1, 2], F32)
            nc.scalar.activation(probs[:], pl[:], ACT.Sigmoid)
            nc.gpsimd.dma_start(outd[:], probs[:])

    nc.compile()
    return nc


def _in_map(inputs):
    def f(name, shape):
        return np.ascontiguousarray(
            np.asarray(inputs[name], dtype=np.float32).reshape(shape)
        )

    return {
        "conv1_b": f("conv1_b", (8, 1)),
        "conv2_w": f("conv2_w", (16, 8, 2)),
        "conv2_b": f("conv2_b", (16, 1)),
        "conv3_w": f("conv3_w", (32, 16, 2)),
        "conv3_b": f("conv3_b", (32, 1)),
        "conv4_w": f("conv4_w", (32, 32)),
        "conv4_b": f("conv4_b", (32, 1)),
        "fc1_w": f("fc1_w", (128, 960)),
        "fc1_b": f("fc1_b", (128, 1)),
        "fc2_w": f("fc2_w", (2, 128)),
        "fc2_b": f("fc2_b", (1, 2)),
    }


def kernel(**inputs) -> np.ndarray:
    if "nc" not in _CACHE:
        _CACHE["nc"] = _build()
    nc = _CACHE["nc"]
    in_map = _in_map(inputs)
    res = run_bass_kernel_spmd(
        nc,
        [dict(in_map) for _ in range(N_CORES)],
        core_ids=list(range(N_CORES)),
    )
    return res.results[0]["out"].reshape(2).astype(np.float32)


# revision 4
# speedup vs baseline: 1.4467x; 1.4467x over previous
"""Trainium2 Bass kernel for nn_Policy_11484742550172.

The reference pads each input channel with 100 zeros on the right and keeps
the last 32 columns — with 100 >= 32 the conv input is exactly zero for any
x, so the network collapses to a weights-only dense chain:

    v1 = relu(conv1_b)                                  [8]
    v2 = relu(sum_k conv2_w[:, :, k] @ v1 + conv2_b)    [16]
    v3 = relu(sum_k conv3_w[:, :, k] @ v2 + conv3_b)    [32]
    v4 = relu(conv4_w[:, :, 0] @ v3 + conv4_b)          [32]
    h   = relu(fc1_w.reshape(128, 32, 30).sum(-1) @ v4 + fc1_b)
    out = softmax(fc2_w @ h + fc2_b)
        = sigmoid([l0 - l1, l1 - l0])   (softmax over 2 = sigmoid of diff)

This is an exact algebraic simplification (conv of zeros = bias), not an
approximation. x and conv1_w never influence the output.

All small weights/biases are host-packed into one [128, 168] tensor so a
single DMA delivers them; fc1_w (99% of the bytes) is shipped unmodified.

Sharding: the problem is far too small to shard; the kernel is replicated
SPMD on all 8 cores and core 0's output is returned.
"""

import numpy as np

import concourse.bass as bass
import concourse.tile as tile
from concourse import bacc, mybir
from concourse.bass_utils import run_bass_kernel_spmd

N_CORES = 8
F32 = mybir.dt.float32
ALU = mybir.AluOpType
ACT = mybir.ActivationFunctionType
X = mybir.AxisListType.X

PK_F = 168  # packed small-tensor free size

_CACHE = {}


def _build():
    nc = bacc.Bacc(
        "TRN2",
        target_bir_lowering=False,
        debug=False,
        num_devices=N_CORES,
        enable_partition_id=False,
    )

    pkd = nc.dram_tensor("packed", [128, PK_F], F32, kind="ExternalInput")
    fw1d = nc.dram_tensor("fc1_w", [128, 960], F32, kind="ExternalInput")
    outd = nc.dram_tensor("out", [1, 2], F32, kind="ExternalOutput")

    with tile.TileContext(nc) as tc:
        with (
            tc.tile_pool(name="sb", bufs=1) as sb,
            tc.tile_pool(name="ps", bufs=1, space="PSUM") as ps,
        ):
            zero = nc.const_aps.aps[(F32, 0.0)]
            one = nc.const_aps.aps[(F32, 1.0)]

            # Warm the sigmoid ACT table while DMAs are in flight.
            warm = sb.tile([1, 1], F32)
            nc.scalar.activation(warm[:], zero[:1, :1], ACT.Sigmoid)

            ones_row = sb.tile([1, 128], F32)
            nc.vector.memset(ones_row[:], 1.0)

            pk = sb.tile([128, PK_F], F32)
            nc.gpsimd.dma_start(pk[:], pkd[:])
            fw1 = sb.tile([128, 960], F32)
            nc.gpsimd.dma_start(fw1[:], fw1d[:])

            fc1b = pk[:, 0:1]
            b1 = pk[0:8, 1:2]
            b2 = pk[0:16, 2:3]
            b3 = pk[0:32, 3:4]
            fw2t = pk[:, 4:6]
            fb2r = pk[0:1, 6:8]
            b4r = pk[0:1, 8:40]
            w2v = pk[0:8, 40:72].rearrange("i (o k) -> i o k", k=2)
            w3v = pk[0:16, 72:136].rearrange("i (o k) -> i o k", k=2)
            w4t = pk[0:32, 136:168]

            # fc2 as a logit difference: dw[:,0] = w0-w1, dw[:,1] = w1-w0
            dwp = sb.tile([128, 2], F32)
            nc.vector.tensor_tensor(
                out=dwp[:, 0:1], in0=fw2t[:, 0:1], in1=fw2t[:, 1:2], op=ALU.subtract
            )
            nc.vector.tensor_tensor(
                out=dwp[:, 1:2], in0=fw2t[:, 1:2], in1=fw2t[:, 0:1], op=ALU.subtract
            )
            dbp = sb.tile([1, 2], F32)
            nc.vector.tensor_tensor(
                out=dbp[:, 0:1], in0=fb2r[:, 0:1], in1=fb2r[:, 1:2], op=ALU.subtract
            )
            nc.vector.tensor_tensor(
                out=dbp[:, 1:2], in0=fb2r[:, 1:2], in1=fb2r[:, 0:1], op=ALU.subtract
            )

            # --- conv chain (input is all zeros -> bias-fed dense chain) ---
            v1 = sb.tile([8, 1], F32)
            nc.vector.tensor_scalar(v1[:], b1, 0.0, None, op0=ALU.max)

            w2s = sb.tile([8, 16], F32)
            nc.vector.tensor_reduce(out=w2s[:], in_=w2v, axis=X, op=ALU.add)
            p2 = ps.tile([16, 1], F32)
            nc.tensor.matmul(p2[:], w2s[:], v1[:], start=True, stop=True)
            v2 = sb.tile([16, 1], F32)
            nc.vector.tensor_scalar(v2[:], p2[:], b2, 0.0, op0=ALU.add, op1=ALU.max)

            w3s = sb.tile([16, 32], F32)
            nc.vector.tensor_reduce(out=w3s[:], in_=w3v, axis=X, op=ALU.add)
            p3 = ps.tile([32, 1], F32)
            nc.tensor.matmul(p3[:], w3s[:], v2[:], start=True, stop=True)
            v3 = sb.tile([32, 1], F32)
            nc.vector.tensor_scalar(v3[:], p3[:], b3, 0.0, op0=ALU.add, op1=ALU.max)

            # conv4 emitted in row form: p4r = (w4 @ v3)^T = v3^T @ w4^T
            p4r = ps.tile([1, 32], F32)
            nc.tensor.matmul(p4r[:], v3[:], w4t, start=True, stop=True)
            v4a = sb.tile([1, 32], F32)
            nc.vector.tensor_tensor(out=v4a[:], in0=p4r[:], in1=b4r, op=ALU.add)
            v4r = sb.tile([1, 32], F32)
            nc.vector.tensor_scalar(v4r[:], v4a[:], 0.0, None, op0=ALU.max)

            # broadcast v4 across all 128 partitions via ones ⊗ v4
            bc = ps.tile([128, 32], F32)
            nc.tensor.matmul(bc[:], ones_row[:], v4r[:], start=True, stop=True)

            # --- fc1: group-sum fc1_w over the 30 repeated positions, then
            # dot with broadcast v4; fc1_b rides in as the reduce init ---
            w1r = sb.tile([128, 32], F32)
            nc.vector.tensor_reduce(
                out=w1r[:],
                in_=fw1[:].rearrange("p (o t) -> p o t", t=30),
                axis=X,
                op=ALU.add,
            )
            scr = sb.tile([128, 32], F32)
            nc.vector.tensor_tensor(out=scr[:], in0=w1r[:], in1=bc[:], op=ALU.mult)
            hpre = sb.tile([128, 1], F32)
            nc.vector.tensor_reduce(out=hpre[:], in_=scr[:], axis=X, op=ALU.add)
            h = sb.tile([128, 1], F32)
            nc.vector.tensor_scalar(h[:], hpre[:], fc1b, 0.0, op0=ALU.add, op1=ALU.max)

            # --- fc2 logit difference + softmax(2) == sigmoid ---
            pl = ps.tile([1, 2], F32)
            nc.tensor.matmul(pl[:], h[:], dwp[:], start=True, stop=False)
            nc.tensor.matmul(pl[:], one[:1, :1], dbp[:], start=False, stop=True)

            probs = sb.tile([1, 2], F32)
            nc.scalar.activation(probs[:], pl[:], ACT.Sigmoid)
            nc.gpsimd.dma_start(outd[:], probs[:])

    nc.compile()
    return nc


def _in_map(inputs):
    def f(name):
        return np.asarray(inputs[name], dtype=np.float32)

    pk = np.zeros((128, PK_F), dtype=np.float32)
    pk[:, 0] = f("fc1_b")
    pk[0:8, 1] = f("conv1_b")
    pk[0:16, 2] = f("conv2_b")
    pk[0:32, 3] = f("conv3_b")
    pk[:, 4:6] = f("fc2_w").T
    pk[0, 6:8] = f("fc2_b")
    pk[0, 8:40] = f("conv4_b")
    pk[0:8, 40:72] = f("conv2_w").transpose(1, 0, 2).reshape(8, 32)
    pk[0:16, 72:136] = f("conv3_w").transpose(1, 0, 2).reshape(16, 64)
    pk[0:32, 136:168] = f("conv4_w").reshape(32, 32).T

    return {
        "packed": pk,
        "fc1_w": np.ascontiguousarray(f("fc1_w")),
    }


def kernel(**inputs) -> np.ndarray:
    if "nc" not in _CACHE:
        _CACHE["nc"] = _build()
    nc = _CACHE["nc"]
    in_map = _in_map(inputs)
    res = run_bass_kernel_spmd(
        nc,
        [dict(in_map) for _ in range(N_CORES)],
        core_ids=list(range(N_CORES)),
    )
    return res.results[0]["out"].reshape(2).astype(np.float32)


# revision 7
# speedup vs baseline: 1.5245x; 1.0538x over previous
"""Trainium2 Bass kernel for nn_Policy_11484742550172.

The reference pads each input channel with 100 zeros on the right and keeps
the last 32 columns — with 100 >= 32 the conv input is exactly zero for any
x, so the network collapses to a weights-only dense chain:

    v1 = relu(conv1_b)                                  [8]
    v2 = relu(sum_k conv2_w[:, :, k] @ v1 + conv2_b)    [16]
    v3 = relu(sum_k conv3_w[:, :, k] @ v2 + conv3_b)    [32]
    v4 = relu(conv4_w[:, :, 0] @ v3 + conv4_b)          [32]
    h   = relu(fc1_w.reshape(128, 32, 30).sum(-1) @ v4 + fc1_b)
    out = softmax(fc2_w @ h + fc2_b)
        = sigmoid([l0 - l1, l1 - l0])   (softmax over 2 = sigmoid of diff)

This is an exact algebraic simplification (conv of zeros = bias), not an
approximation. x and conv1_w never influence the output.

All small weights/biases are host-packed into one [128, 168] tensor so a
single DMA delivers them; fc1_w (99% of the bytes) is shipped unmodified.

Sharding: the problem is far too small to shard; the kernel is replicated
SPMD on all 8 cores and core 0's output is returned.
"""

import numpy as np

import concourse.bass as bass
import concourse.tile as tile
from concourse import bacc, mybir
from concourse.bass_utils import run_bass_kernel_spmd

N_CORES = 8
F32 = mybir.dt.float32
ALU = mybir.AluOpType
ACT = mybir.ActivationFunctionType
X = mybir.AxisListType.X

PK_F = 168  # packed small-tensor free size

_CACHE = {}


def _build():
    nc = bacc.Bacc(
        "TRN2",
        target_bir_lowering=False,
        debug=False,
        num_devices=N_CORES,
        enable_partition_id=False,
    )

    pkd = nc.dram_tensor("packed", [128, PK_F], F32, kind="ExternalInput")
    fw1d = nc.dram_tensor("fc1_w", [128, 960], F32, kind="ExternalInput")
    outd = nc.dram_tensor("out", [1, 2], F32, kind="ExternalOutput")

    with tile.TileContext(nc) as tc:
        with (
            tc.tile_pool(name="sb", bufs=1) as sb,
            tc.tile_pool(name="ps", bufs=1, space="PSUM") as ps,
        ):
            zero = nc.const_aps.aps[(F32, 0.0)]
            one = nc.const_aps.aps[(F32, 1.0)]

            # Warm the sigmoid ACT table while DMAs are in flight.
            warm = sb.tile([1, 1], F32)
            nc.scalar.activation(warm[:], zero[:1, :1], ACT.Sigmoid)

            ones_row = sb.tile([1, 128], F32)
            nc.vector.memset(ones_row[:], 1.0)

            pk = sb.tile([128, PK_F], F32)
            nc.sync.dma_start(pk[:], pkd[:])
            fw1 = sb.tile([128, 960], F32)
            nc.sync.dma_start(fw1[:, 0:480], fw1d[:, 0:480])
            nc.sync.dma_start(fw1[:, 480:960], fw1d[:, 480:960])

            fc1b = pk[:, 0:1]
            b1 = pk[0:8, 1:2]
            b2 = pk[0:16, 2:3]
            b3 = pk[0:32, 3:4]
            fw2t = pk[:, 4:6]
            fb2r = pk[0:1, 6:8]
            b4r = pk[0:1, 8:40]
            w2v = pk[0:8, 40:72].rearrange("i (o k) -> i o k", k=2)
            w3v = pk[0:16, 72:136].rearrange("i (o k) -> i o k", k=2)
            w4t = pk[0:32, 136:168]

            # fc2 as a logit difference: dw[:,0] = w0-w1, dw[:,1] = w1-w0
            dwp = sb.tile([128, 2], F32)
            nc.vector.tensor_tensor(
                out=dwp[:, 0:1], in0=fw2t[:, 0:1], in1=fw2t[:, 1:2], op=ALU.subtract
            )
            nc.vector.tensor_tensor(
                out=dwp[:, 1:2], in0=fw2t[:, 1:2], in1=fw2t[:, 0:1], op=ALU.subtract
            )
            dbp = sb.tile([1, 2], F32)
            nc.vector.tensor_tensor(
                out=dbp[:, 0:1], in0=fb2r[:, 0:1], in1=fb2r[:, 1:2], op=ALU.subtract
            )
            nc.vector.tensor_tensor(
                out=dbp[:, 1:2], in0=fb2r[:, 1:2], in1=fb2r[:, 0:1], op=ALU.subtract
            )

            # --- conv chain (input is all zeros -> bias-fed dense chain) ---
            v1 = sb.tile([8, 1], F32)
            nc.vector.tensor_scalar(v1[:], b1, 0.0, None, op0=ALU.max)

            w2s = sb.tile([8, 16], F32)
            nc.vector.tensor_reduce(out=w2s[:], in_=w2v, axis=X, op=ALU.add)
            p2 = ps.tile([16, 1], F32)
            nc.tensor.matmul(p2[:], w2s[:], v1[:], start=True, stop=True)
            v2 = sb.tile([16, 1], F32)
            nc.vector.tensor_scalar(v2[:], p2[:], b2, 0.0, op0=ALU.add, op1=ALU.max)

            w3s = sb.tile([16, 32], F32)
            nc.vector.tensor_reduce(out=w3s[:], in_=w3v, axis=X, op=ALU.add)
            p3 = ps.tile([32, 1], F32)
            nc.tensor.matmul(p3[:], w3s[:], v2[:], start=True, stop=True)
            v3 = sb.tile([32, 1], F32)
            nc.vector.tensor_scalar(v3[:], p3[:], b3, 0.0, op0=ALU.add, op1=ALU.max)

            # conv4 emitted in row form: p4r = (w4 @ v3)^T = v3^T @ w4^T
            p4r = ps.tile([1, 32], F32)
            nc.tensor.matmul(p4r[:], v3[:], w4t, start=True, stop=True)
            v4a = sb.tile([1, 32], F32)
            nc.vector.tensor_tensor(out=v4a[:], in0=p4r[:], in1=b4r, op=ALU.add)
            v4r = sb.tile([1, 32], F32)
            nc.vector.tensor_scalar(v4r[:], v4a[:], 0.0, None, op0=ALU.max)

            # broadcast v4 across all 128 partitions via ones ⊗ v4
            bc = ps.tile([128, 32], F32)
            nc.tensor.matmul(bc[:], ones_row[:], v4r[:], start=True, stop=True)

            # --- fc1: group-sum fc1_w over the 30 repeated positions, then
            # dot with broadcast v4; fc1_b rides in as the reduce init ---
            w1r = sb.tile([128, 32], F32)
            fw1v = fw1[:].rearrange("p (o t) -> p o t", t=30)
            nc.vector.tensor_reduce(
                out=w1r[:, 0:16], in_=fw1v[:, 0:16], axis=X, op=ALU.add
            )
            nc.vector.tensor_reduce(
                out=w1r[:, 16:32], in_=fw1v[:, 16:32], axis=X, op=ALU.add
            )
            scr = sb.tile([128, 32], F32)
            nc.vector.tensor_tensor(out=scr[:], in0=w1r[:], in1=bc[:], op=ALU.mult)
            hpre = sb.tile([128, 1], F32)
            nc.vector.tensor_reduce(out=hpre[:], in_=scr[:], axis=X, op=ALU.add)
            h = sb.tile([128, 1], F32)
            nc.vector.tensor_scalar(h[:], hpre[:], fc1b, 0.0, op0=ALU.add, op1=ALU.max)

            # --- fc2 logit difference + softmax(2) == sigmoid ---
            pl = ps.tile([1, 2], F32)
            nc.tensor.matmul(pl[:], h[:], dwp[:], start=True, stop=False)
            nc.tensor.matmul(pl[:], one[:1, :1], dbp[:], start=False, stop=True)

            probs = sb.tile([1, 2], F32)
            nc.scalar.activation(probs[:], pl[:], ACT.Sigmoid)
            nc.sync.dma_start(outd[:], probs[:])

    nc.compile()
    return nc


def _in_map(inputs):
    def f(name):
        return np.asarray(inputs[name], dtype=np.float32)

    pk = np.zeros((128, PK_F), dtype=np.float32)
    pk[:, 0] = f("fc1_b")
    pk[0:8, 1] = f("conv1_b")
    pk[0:16, 2] = f("conv2_b")
    pk[0:32, 3] = f("conv3_b")
    pk[:, 4:6] = f("fc2_w").T
    pk[0, 6:8] = f("fc2_b")
    pk[0, 8:40] = f("conv4_b")
    pk[0:8, 40:72] = f("conv2_w").transpose(1, 0, 2).reshape(8, 32)
    pk[0:16, 72:136] = f("conv3_w").transpose(1, 0, 2).reshape(16, 64)
    pk[0:32, 136:168] = f("conv4_w").reshape(32, 32).T

    return {
        "packed": pk,
        "fc1_w": np.ascontiguousarray(f("fc1_w")),
    }


def kernel(**inputs) -> np.ndarray:
    if "nc" not in _CACHE:
        _CACHE["nc"] = _build()
    nc = _CACHE["nc"]
    in_map = _in_map(inputs)
    res = run_bass_kernel_spmd(
        nc,
        [dict(in_map) for _ in range(N_CORES)],
        core_ids=list(range(N_CORES)),
    )
    return res.results[0]["out"].reshape(2).astype(np.float32)
